# revision 8
# baseline (speedup 1.0000x reference)
"""ChebConv (K=4) Trainium2 kernel — sparse gather-based SpMM.

Math (exactly matches the reference, which applies the spmm to `x` — not the
recurrence state — in every Chebyshev iteration):

    deg   = segment_sum(edge_weight, row)
    dinv  = deg^-1/2 (0 where deg <= 0)
    lap_e = -2*dinv[row]*edge_weight*dinv[col]      (edge part of L)
    Lx    = lap_e-SpMM(x) + 2*fill*x                (self loops)
    out   = x @ (W0 - W2) + Lx @ (W1 + 2*W2 + W3) + bias
          = x @ A' + (lap_e-SpMM(x)) @ B + bias,
            A' = W0 - W2 + 2*fill*B,  B = W1 + 2*W2 + W3

Device strategy: shard the 10240 (padded) destination rows over 8 cores
(10 dest-tiles of 128 rows each per core).  Per dest-tile, the ~2000 in-edges
are gathered edge-major with dma_gather (source-node feature rows of
512 bf16 = 1KB each, indices wrapped i%128 -> partition), then NS scatter
matmuls S_j^T @ gathered_j accumulate the tile's Lx in PSUM, where
S_j[e,d] = lap_e one-hot-by-dest (built on host, streamed bf16).  This
replaces the dense (1280 x 10240) @ (10240 x 512) SpMM: PE work drops
~171us -> ~40us and the kernel becomes DMA-bound (~30MB/core).
Phase 1.5/2 transpose Lx per tile and apply the feature transforms in bf16.
"""

import numpy as np
import ml_dtypes

B = 4
N_NODES = 10000
F = 128
SELF_LOOP_FILL = -0.05
NCORES = 8
NPAD = 10240                 # 80 tiles of 128; divisible by 8 cores
MROWS = NPAD // NCORES       # 1280 output rows per core
MT = MROWS // 128            # 10 dest-tiles per core
GT = NPAD // 128             # 80 global dest-tiles
BF = B * F                   # 512 features per gathered node row

_state = {}


def _build_nc(NS):
    """NS = slot-tiles (groups of 128 gathered edges) per dest-tile."""
    from contextlib import ExitStack

    import concourse.bass as bass
    import concourse.bacc as bacc
    import concourse.tile as tile
    from concourse import mybir

    dt = mybir.dt
    NS8 = NS * 8            # int16 idx columns per dest-tile (16-wrap)
    NI = NS * 128           # gathered rows per dest-tile
    nc = bacc.Bacc(
        "TRN2", target_bir_lowering=False, debug=False, num_devices=NCORES
    )

    xsrc = nc.declare_dram_parameter("xsrc", [N_NODES, BF], dt.bfloat16, isOutput=False)
    smat = nc.declare_dram_parameter(
        "smat", [128, MT * NI], dt.bfloat16, isOutput=False
    )
    idxm = nc.declare_dram_parameter("idxm", [128, MT * NS8], dt.int16, isOutput=False)
    xt = nc.declare_dram_parameter("xt", [128, B, MROWS], dt.bfloat16, isOutput=False)
    wa = nc.declare_dram_parameter("wa", [128, 128], dt.bfloat16, isOutput=False)
    wb = nc.declare_dram_parameter("wb", [128, 128], dt.bfloat16, isOutput=False)
    biasv = nc.declare_dram_parameter("biasv", [128, 1], dt.float32, isOutput=False)
    ident = nc.declare_dram_parameter("ident", [128, 128], dt.float32, isOutput=False)
    out_t = nc.declare_dram_parameter("out_t", [B, 128, MROWS], dt.float32, isOutput=True)

    from concourse import library_config

    with ExitStack() as ctx:
        tc = ctx.enter_context(tile.TileContext(nc))
        nc.gpsimd.load_library(library_config.mlp)
        const = ctx.enter_context(tc.tile_pool(name="const", bufs=1))
        xgpool = ctx.enter_context(tc.tile_pool(name="xg", bufs=3))
        spool = ctx.enter_context(tc.tile_pool(name="smat", bufs=3))
        lxpool = ctx.enter_context(tc.tile_pool(name="lx", bufs=2))
        lxtpool = ctx.enter_context(tc.tile_pool(name="lxt", bufs=2))
        outpool = ctx.enter_context(tc.tile_pool(name="outstg", bufs=3))
        psum = ctx.enter_context(
            tc.tile_pool(name="psum", bufs=8, space=bass.MemorySpace.PSUM)
        )

        # constants on the scalar HWDGE queue; idx table on sync (the first
        # gather depends on it — issue early)
        idx_sb = const.tile([128, MT * NS8], dt.int16, tag="idx")
        nc.sync.dma_start(idx_sb[:], idxm[:])
        id_sb = const.tile([128, 128], dt.float32, tag="ident")
        nc.scalar.dma_start(id_sb[:], ident[:])
        wa_sb = const.tile([128, 128], dt.bfloat16, tag="wa")
        nc.scalar.dma_start(wa_sb[:], wa[:])
        wb_sb = const.tile([128, 128], dt.bfloat16, tag="wb")
        nc.scalar.dma_start(wb_sb[:], wb[:])
        bias_sb = const.tile([128, 1], dt.float32, tag="bias")
        nc.scalar.dma_start(bias_sb[:], biasv[:])
        xt_sb = const.tile([128, B, MROWS], dt.bfloat16, tag="xt")
        nc.scalar.dma_start(xt_sb[:], xt[:])

        # PE warmup: dummy matmuls on the identity so the HAM clock-gate and
        # p-state ramp open before the first real gather lands.
        pw = psum.tile([128, 128], dt.float32, tag="ps", name="ps_warm")
        for i in range(36):
            nc.tensor.matmul(
                pw[:], id_sb[:], id_sb[:], start=(i == 0), stop=(i == 35)
            )

        for t in range(MT):
            # gather the tile's edge source rows: [128 e-lane, NS slot, 512]
            xg = xgpool.tile([128, NS, BF], dt.bfloat16, tag="xg")
            nc.gpsimd.dma_gather(
                xg[:], xsrc[:], idx_sb[:, t * NS8 : (t + 1) * NS8], NI, NI, BF,
                single_packet=False,
            )
            st = spool.tile([128, NS * 128], dt.bfloat16, tag="st")
            nc.sync.dma_start(st[:], smat[:, t * NI : (t + 1) * NI])

            ps = psum.tile([128, BF], dt.float32, tag="ps", name=f"ps1_{t}")
            for j in range(NS):
                nc.tensor.matmul(
                    ps[:],
                    st[:, j * 128 : (j + 1) * 128],
                    xg[:, j, :],
                    start=(j == 0),
                    stop=(j == NS - 1),
                )
            lx = lxpool.tile([128, BF], dt.float32, tag="lx")
            nc.vector.tensor_copy(lx[:], ps[:])

            # transpose Lx tile (node-major -> feature-major), per batch
            pt = psum.tile([128, BF], dt.float32, tag="ps", name=f"pt_{t}")
            for b in range(B):
                nc.tensor.transpose(
                    pt[:, b * 128 : (b + 1) * 128],
                    lx[:, b * 128 : (b + 1) * 128],
                    id_sb[:],
                )
            lxT = lxtpool.tile([128, BF], dt.bfloat16, tag="lxT")
            nc.vector.tensor_copy(lxT[:], pt[:])

            # phase 2: out_T = A'^T x^T + B^T Lx^T + bias (bf16 in, fp32 out)
            ps2 = psum.tile([128, BF], dt.float32, tag="ps", name=f"ps2_{t}")
            for b in range(B):
                nc.tensor.matmul(
                    ps2[:, b * 128 : (b + 1) * 128],
                    wa_sb[:],
                    xt_sb[:, b, t * 128 : (t + 1) * 128],
                    start=True, stop=False,
                )
                nc.tensor.matmul(
                    ps2[:, b * 128 : (b + 1) * 128],
                    wb_sb[:],
                    lxT[:, b * 128 : (b + 1) * 128],
                    start=False, stop=True,
                )
            ot = outpool.tile([128, BF], dt.float32, tag="ot")
            nc.scalar.activation(
                ot[:], ps2[:],
                mybir.ActivationFunctionType.Identity,
                bias=bias_sb[:],
            )
            for b in range(B):
                nc.scalar.dma_start(
                    out_t[b, :, t * 128 : (t + 1) * 128],
                    ot[:, b * 128 : (b + 1) * 128],
                )

    return nc


def _get_nc(NS):
    key = ("nc", NS)
    if key not in _state:
        nc = _build_nc(NS)
        nc.compile()
        _state[key] = nc
    return _state[key]


def _prep_inputs(x, edge_index, edge_weight, weight, bias):
    """Host-side graph preprocessing -> per-core device input maps + NS."""
    bf16 = ml_dtypes.bfloat16
    row = np.asarray(edge_index[0], dtype=np.int64)
    col = np.asarray(edge_index[1], dtype=np.int64)
    w = np.asarray(edge_weight, dtype=np.float32)

    deg = np.bincount(row, weights=w.astype(np.float64), minlength=N_NODES)
    deg = deg.astype(np.float32)
    dinv = np.where(deg > 0, np.where(deg > 0, deg, 1.0) ** -0.5, 0.0).astype(
        np.float32
    )
    lap2 = (-2.0 * dinv[row] * w * dinv[col]).astype(np.float32)

    # group edges by global dest-tile (row // 128)
    gt = row // 128
    order = np.argsort(gt, kind="stable")
    gts = gt[order]
    counts = np.bincount(gts, minlength=GT)
    starts = np.concatenate([[0], np.cumsum(counts)[:-1]])
    pos = np.arange(len(gts)) - starts[gts]          # position within tile
    NS = int(np.max((counts + 127) // 128))
    NS = max(NS, 1)
    NS8 = NS * 8
    NI = NS * 128

    e_col = col[order]
    e_lap = lap2[order]
    e_lane = pos % 128
    e_slot = pos // 128
    e_dst = (row[order] % 128).astype(np.int64)

    # per-tile padded index lists (pad with node 0; S row stays 0)
    idx_full = np.zeros((GT, NI), dtype=np.int16)
    idx_full[gts, pos] = e_col.astype(np.int16)
    # scatter matrices: smat_all[gt, lane, slot*128 + dst] = lap
    smat_all = np.zeros((GT, 128, NI), dtype=np.float32)
    smat_all[gts, e_lane, e_slot * 128 + e_dst] = e_lap
    smat_all = smat_all.astype(bf16)

    # x in node-major (node, batch*feat) bf16 — the gather source
    xn = np.ascontiguousarray(
        np.transpose(np.asarray(x, np.float32), (1, 0, 2)).reshape(N_NODES, BF)
    )
    xsrc = xn.astype(bf16)
    xn_pad = np.zeros((NPAD, BF), dtype=np.float32)
    xn_pad[:N_NODES] = xn

    W = np.asarray(weight, dtype=np.float32)
    Bm = W[1] + 2.0 * W[2] + W[3]
    A = W[0] - W[2] + 2.0 * SELF_LOOP_FILL * Bm
    biasv = np.asarray(bias, dtype=np.float32).reshape(128, 1)
    identity = np.eye(128, dtype=np.float32)

    in_maps = []
    for c in range(NCORES):
        t0 = c * MT
        # idx wrap: idxm[p, t*NS8 + s] = idx_full[t0+t, s*16 + p%16]
        blk = idx_full[t0 : t0 + MT].reshape(MT, NS8, 16)       # (MT, s, 16)
        idxm16 = blk.transpose(2, 0, 1).reshape(16, MT * NS8)   # (16, MT*NS8)
        idxm = np.ascontiguousarray(np.tile(idxm16, (8, 1)))    # replicate
        smat = np.ascontiguousarray(
            smat_all[t0 : t0 + MT].transpose(1, 0, 2).reshape(128, MT * NI)
        )
        r0 = c * MROWS
        xtc = np.ascontiguousarray(
            xn_pad[r0 : r0 + MROWS].reshape(MROWS, B, F).transpose(2, 1, 0)
        ).astype(bf16)
        in_maps.append(
            {
                "xsrc": xsrc,
                "smat": smat,
                "idxm": idxm,
                "xt": xtc,
                "wa": A.astype(bf16),
                "wb": Bm.astype(bf16),
                "biasv": biasv,
                "ident": identity,
            }
        )
    return in_maps, NS


def _ensure_ntff_hook():
    """Register the axon NTFF profiling hook if the image's antenv lacks it.

    The boot path degrades silently when ``antenv.axon_hooks`` is missing;
    recreate the tiny get/set holder and wire it to libaxon_pjrt.so so
    ``run_bass_kernel_spmd(trace=True)`` can capture NTFF profiles.
    """
    import sys
    import types

    try:
        from antenv.axon_hooks import get_axon_ntff_profile_hook  # noqa: F401

        return
    except ImportError:
        pass
    mod = types.ModuleType("antenv.axon_hooks")
    holder = {}
    mod.set_axon_ntff_profile_hook = lambda h: holder.__setitem__("h", h)
    mod.get_axon_ntff_profile_hook = lambda: holder.get("h")
    sys.modules["antenv.axon_hooks"] = mod
    import antenv

    antenv.axon_hooks = mod
    from trn_agent_boot.trn_boot import _ntff_profile_via_ctypes

    hook = _ntff_profile_via_ctypes("/opt/axon/libaxon_pjrt.so")
    if hook is not None:
        mod.set_axon_ntff_profile_hook(hook)


def kernel(x, edge_index, edge_weight, weight, bias):
    import os

    from concourse.bass_utils import run_bass_kernel_spmd

    x = np.asarray(x, dtype=np.float32)
    in_maps, NS = _prep_inputs(x, edge_index, edge_weight, weight, bias)
    nc = _get_nc(NS)
    trace = bool(int(os.environ.get("CHEB_TRACE", "0")))
    if trace:
        _ensure_ntff_hook()
    res = run_bass_kernel_spmd(nc, in_maps, list(range(NCORES)), trace=trace)
    _state["last_result"] = res
    out_T = np.concatenate([res.results[c]["out_t"] for c in range(NCORES)], axis=2)
    out = np.ascontiguousarray(out_T.transpose(0, 2, 1)[:, :N_NODES, :])
    return out


# revision 10
# speedup vs baseline: 1.4354x; 1.4354x over previous
"""ChebConv (K=4) Trainium2 kernel — sparse gather-based SpMM.

Math (exactly matches the reference, which applies the spmm to `x` — not the
recurrence state — in every Chebyshev iteration):

    deg   = segment_sum(edge_weight, row)
    dinv  = deg^-1/2 (0 where deg <= 0)
    lap_e = -2*dinv[row]*edge_weight*dinv[col]      (edge part of L)
    Lx    = lap_e-SpMM(x) + 2*fill*x                (self loops)
    out   = x @ (W0 - W2) + Lx @ (W1 + 2*W2 + W3) + bias
          = x @ A' + (lap_e-SpMM(x)) @ B + bias,
            A' = W0 - W2 + 2*fill*B,  B = W1 + 2*W2 + W3

Device strategy: shard the 10240 (padded) destination rows over 8 cores
(10 dest-tiles of 128 rows each per core).  Per dest-tile, the ~2000 in-edges
are gathered edge-major with dma_gather (source-node rows of 512 bf16 = 1KB,
index i -> partition i%128), in 384-index single-packet chunks rotated over
the 4 SWDGE queues so Q7 descriptor generation overlaps SDMA drain.  NS
scatter matmuls S_j^T @ gathered_j accumulate the tile's Lx in PSUM; the
one-hot-by-dest S_j (lap values) are built on the otherwise-idle DVE from a
compact (dest,lap) table via tensor_scalar(iota is_equal dest) * lap.
Phase 1.5/2 transpose Lx per tile and apply the feature transforms in bf16;
outputs are written bf16 and upcast on host.
"""

import numpy as np
import ml_dtypes

B = 4
N_NODES = 10000
F = 128
SELF_LOOP_FILL = -0.05
NCORES = 8
NPAD = 10240                 # 80 tiles of 128; divisible by 8 cores
MROWS = NPAD // NCORES       # 1280 output rows per core
MT = MROWS // 128            # 10 dest-tiles per core
GT = NPAD // 128             # 80 global dest-tiles
BF = B * F                   # 512 features per gathered node row
CSLOT = 3                    # slot-tiles per gather chunk (384 idxs)
NQ = 4                       # SWDGE queues to rotate gather chunks over

_state = {}


def _build_nc(NS):
    """NS = slot-tiles (groups of 128 gathered edges) per dest-tile."""
    from contextlib import ExitStack

    import concourse.bass as bass
    import concourse.bacc as bacc
    import concourse.tile as tile
    from concourse import mybir, library_config

    dt = mybir.dt
    NS8 = NS * 8            # int16 idx columns per dest-tile (16-wrap)
    NI = NS * 128           # gathered rows per dest-tile
    nc = bacc.Bacc(
        "TRN2", target_bir_lowering=False, debug=False, num_devices=NCORES,
        num_swdge_queues=NQ,
    )

    xsrc = nc.declare_dram_parameter("xsrc", [N_NODES, BF], dt.bfloat16, isOutput=False)
    dstm = nc.declare_dram_parameter("dstm", [128, MT * NS], dt.float32, isOutput=False)
    lapm = nc.declare_dram_parameter("lapm", [128, MT * NS], dt.float32, isOutput=False)
    idxm = nc.declare_dram_parameter("idxm", [128, MT * NS8], dt.int16, isOutput=False)
    xt = nc.declare_dram_parameter("xt", [128, B, MROWS], dt.bfloat16, isOutput=False)
    wa = nc.declare_dram_parameter("wa", [128, 128], dt.bfloat16, isOutput=False)
    wb = nc.declare_dram_parameter("wb", [128, 128], dt.bfloat16, isOutput=False)
    biasv = nc.declare_dram_parameter("biasv", [128, 1], dt.float32, isOutput=False)
    ident = nc.declare_dram_parameter("ident", [128, 128], dt.float32, isOutput=False)
    iota = nc.declare_dram_parameter("iota", [128, 128], dt.bfloat16, isOutput=False)
    out_t = nc.declare_dram_parameter(
        "out_t", [B, 128, MROWS], dt.bfloat16, isOutput=True
    )

    with ExitStack() as ctx:
        tc = ctx.enter_context(tile.TileContext(nc))
        nc.gpsimd.load_library(library_config.mlp)
        const = ctx.enter_context(tc.tile_pool(name="const", bufs=1))
        xgpool = ctx.enter_context(tc.tile_pool(name="xg", bufs=3))
        spool = ctx.enter_context(tc.tile_pool(name="smat", bufs=3))
        lxpool = ctx.enter_context(tc.tile_pool(name="lx", bufs=2))
        lxtpool = ctx.enter_context(tc.tile_pool(name="lxt", bufs=2))
        outpool = ctx.enter_context(tc.tile_pool(name="outstg", bufs=3))
        psum = ctx.enter_context(
            tc.tile_pool(name="psum", bufs=8, space=bass.MemorySpace.PSUM)
        )

        # idx table on sync (the first gather depends on it — issue early);
        # other constants on the scalar HWDGE queue
        idx_sb = const.tile([128, MT * NS8], dt.int16, tag="idx")
        nc.sync.dma_start(idx_sb[:], idxm[:])
        dst_sb = const.tile([128, MT * NS], dt.float32, tag="dst")
        nc.sync.dma_start(dst_sb[:], dstm[:])
        lap_sb = const.tile([128, MT * NS], dt.float32, tag="lap")
        nc.sync.dma_start(lap_sb[:], lapm[:])
        iota_sb = const.tile([128, 128], dt.bfloat16, tag="iota")
        nc.scalar.dma_start(iota_sb[:], iota[:])
        id_sb = const.tile([128, 128], dt.float32, tag="ident")
        nc.scalar.dma_start(id_sb[:], ident[:])
        wa_sb = const.tile([128, 128], dt.bfloat16, tag="wa")
        nc.scalar.dma_start(wa_sb[:], wa[:])
        wb_sb = const.tile([128, 128], dt.bfloat16, tag="wb")
        nc.scalar.dma_start(wb_sb[:], wb[:])
        bias_sb = const.tile([128, 1], dt.float32, tag="bias")
        nc.scalar.dma_start(bias_sb[:], biasv[:])
        xt_sb = const.tile([128, B, MROWS], dt.bfloat16, tag="xt")
        nc.scalar.dma_start(xt_sb[:], xt[:])

        # PE warmup: dummy matmuls on the identity so the HAM clock-gate and
        # p-state ramp open before the first real gather lands.
        pw = psum.tile([128, 128], dt.float32, tag="ps", name="ps_warm")
        for i in range(36):
            nc.tensor.matmul(
                pw[:], id_sb[:], id_sb[:], start=(i == 0), stop=(i == 35)
            )

        q = 0
        for t in range(MT):
            # gather the tile's edge source rows: [128 e-lane, NS slot, 512],
            # in CSLOT-sized chunks rotating over the SWDGE queues
            xg = xgpool.tile([128, NS, BF], dt.bfloat16, tag="xg")
            for c0 in range(0, NS, CSLOT):
                cw = min(CSLOT, NS - c0)
                nc.gpsimd.dma_gather(
                    xg[:, c0 : c0 + cw, :],
                    xsrc[:],
                    idx_sb[:, t * NS8 + c0 * 8 : t * NS8 + (c0 + cw) * 8],
                    cw * 128, cw * 128, BF,
                    single_packet=True,
                    queue_num=q,
                )
                q = (q + 1) % NQ

            # build the scatter matrices on DVE: S_j[e,d] = lap * (d == dest)
            st = spool.tile([128, NS * 128], dt.bfloat16, tag="st")
            for j in range(NS):
                nc.vector.tensor_scalar(
                    st[:, j * 128 : (j + 1) * 128],
                    iota_sb[:],
                    dst_sb[:, t * NS + j : t * NS + j + 1],
                    lap_sb[:, t * NS + j : t * NS + j + 1],
                    mybir.AluOpType.is_equal,
                    mybir.AluOpType.mult,
                )

            ps = psum.tile([128, BF], dt.float32, tag="ps", name=f"ps1_{t}")
            for j in range(NS):
                nc.tensor.matmul(
                    ps[:],
                    st[:, j * 128 : (j + 1) * 128],
                    xg[:, j, :],
                    start=(j == 0),
                    stop=(j == NS - 1),
                )
            lx = lxpool.tile([128, BF], dt.float32, tag="lx")
            nc.vector.tensor_copy(lx[:], ps[:])

            # transpose Lx tile (node-major -> feature-major), per batch
            pt = psum.tile([128, BF], dt.float32, tag="ps", name=f"pt_{t}")
            for b in range(B):
                nc.tensor.transpose(
                    pt[:, b * 128 : (b + 1) * 128],
                    lx[:, b * 128 : (b + 1) * 128],
                    id_sb[:],
                )
            lxT = lxtpool.tile([128, BF], dt.bfloat16, tag="lxT")
            nc.vector.tensor_copy(lxT[:], pt[:])

            # phase 2: out_T = A'^T x^T + B^T Lx^T + bias (bf16 in, fp32 psum)
            ps2 = psum.tile([128, BF], dt.float32, tag="ps", name=f"ps2_{t}")
            for b in range(B):
                nc.tensor.matmul(
                    ps2[:, b * 128 : (b + 1) * 128],
                    wa_sb[:],
                    xt_sb[:, b, t * 128 : (t + 1) * 128],
                    start=True, stop=False,
                )
                nc.tensor.matmul(
                    ps2[:, b * 128 : (b + 1) * 128],
                    wb_sb[:],
                    lxT[:, b * 128 : (b + 1) * 128],
                    start=False, stop=True,
                )
            ot = outpool.tile([128, BF], dt.bfloat16, tag="ot")
            nc.scalar.activation(
                ot[:], ps2[:],
                mybir.ActivationFunctionType.Identity,
                bias=bias_sb[:],
            )
            for b in range(B):
                nc.scalar.dma_start(
                    out_t[b, :, t * 128 : (t + 1) * 128],
                    ot[:, b * 128 : (b + 1) * 128],
                )

    return nc


def _get_nc(NS):
    key = ("nc", NS)
    if key not in _state:
        nc = _build_nc(NS)
        nc.compile()
        _state[key] = nc
    return _state[key]


def _prep_inputs(x, edge_index, edge_weight, weight, bias):
    """Host-side graph preprocessing -> per-core device input maps + NS."""
    bf16 = ml_dtypes.bfloat16
    row = np.asarray(edge_index[0], dtype=np.int64)
    col = np.asarray(edge_index[1], dtype=np.int64)
    w = np.asarray(edge_weight, dtype=np.float32)

    deg = np.bincount(row, weights=w.astype(np.float64), minlength=N_NODES)
    deg = deg.astype(np.float32)
    dinv = np.where(deg > 0, np.where(deg > 0, deg, 1.0) ** -0.5, 0.0).astype(
        np.float32
    )
    lap2 = (-2.0 * dinv[row] * w * dinv[col]).astype(np.float32)

    # group edges by global dest-tile (row // 128)
    gt = row // 128
    order = np.argsort(gt, kind="stable")
    gts = gt[order]
    counts = np.bincount(gts, minlength=GT)
    starts = np.concatenate([[0], np.cumsum(counts)[:-1]])
    pos = np.arange(len(gts)) - starts[gts]          # position within tile
    NS = int(np.max((counts + 127) // 128))
    NS = max(NS, 1)
    NS8 = NS * 8
    NI = NS * 128

    e_col = col[order]
    e_lap = lap2[order]
    e_lane = pos % 128
    e_slot = pos // 128
    e_dst = (row[order] % 128).astype(np.int64)

    # per-tile padded index lists (pad with node 0; lap stays 0 -> S row 0)
    idx_full = np.zeros((GT, NI), dtype=np.int16)
    idx_full[gts, pos] = e_col.astype(np.int16)
    # compact scatter-matrix tables: dest lane + lap value per (tile, slot, lane)
    dst_all = np.zeros((GT, 128, NS), dtype=np.float32)
    lap_all = np.zeros((GT, 128, NS), dtype=np.float32)
    dst_all[gts, e_lane, e_slot] = e_dst.astype(np.float32)
    # padded lanes: dest 0 with lap 0 (contributes nothing)
    lap_all[gts, e_lane, e_slot] = e_lap

    # x in node-major (node, batch*feat) bf16 — the gather source
    xn = np.ascontiguousarray(
        np.transpose(np.asarray(x, np.float32), (1, 0, 2)).reshape(N_NODES, BF)
    )
    xsrc = xn.astype(bf16)
    xn_pad = np.zeros((NPAD, BF), dtype=np.float32)
    xn_pad[:N_NODES] = xn

    W = np.asarray(weight, dtype=np.float32)
    Bm = W[1] + 2.0 * W[2] + W[3]
    A = W[0] - W[2] + 2.0 * SELF_LOOP_FILL * Bm
    biasv = np.asarray(bias, dtype=np.float32).reshape(128, 1)
    identity = np.eye(128, dtype=np.float32)
    iota = np.broadcast_to(np.arange(128, dtype=np.float32), (128, 128))

    in_maps = []
    for c in range(NCORES):
        t0 = c * MT
        # idx wrap: idxm[p, t*NS8 + s] = idx_full[t0+t, s*16 + p%16]
        blk = idx_full[t0 : t0 + MT].reshape(MT, NS8, 16)       # (MT, s, 16)
        idxm16 = blk.transpose(2, 0, 1).reshape(16, MT * NS8)   # (16, MT*NS8)
        idxm = np.ascontiguousarray(np.tile(idxm16, (8, 1)))    # replicate
        dstm = np.ascontiguousarray(
            dst_all[t0 : t0 + MT].transpose(1, 0, 2).reshape(128, MT * NS)
        )
        lapm = np.ascontiguousarray(
            lap_all[t0 : t0 + MT].transpose(1, 0, 2).reshape(128, MT * NS)
        )
        r0 = c * MROWS
        xtc = np.ascontiguousarray(
            xn_pad[r0 : r0 + MROWS].reshape(MROWS, B, F).transpose(2, 1, 0)
        ).astype(bf16)
        in_maps.append(
            {
                "xsrc": xsrc,
                "dstm": dstm,
                "lapm": lapm,
                "idxm": idxm,
                "xt": xtc,
                "wa": A.astype(bf16),
                "wb": Bm.astype(bf16),
                "biasv": biasv,
                "ident": identity,
                "iota": np.ascontiguousarray(iota).astype(bf16),
            }
        )
    return in_maps, NS


def _ensure_ntff_hook():
    """Register the axon NTFF profiling hook if the image's antenv lacks it.

    The boot path degrades silently when ``antenv.axon_hooks`` is missing;
    recreate the tiny get/set holder and wire it to libaxon_pjrt.so so
    ``run_bass_kernel_spmd(trace=True)`` can capture NTFF profiles.
    """
    import sys
    import types

    try:
        from antenv.axon_hooks import get_axon_ntff_profile_hook  # noqa: F401

        return
    except ImportError:
        pass
    mod = types.ModuleType("antenv.axon_hooks")
    holder = {}
    mod.set_axon_ntff_profile_hook = lambda h: holder.__setitem__("h", h)
    mod.get_axon_ntff_profile_hook = lambda: holder.get("h")
    sys.modules["antenv.axon_hooks"] = mod
    import antenv

    antenv.axon_hooks = mod
    from trn_agent_boot.trn_boot import _ntff_profile_via_ctypes

    hook = _ntff_profile_via_ctypes("/opt/axon/libaxon_pjrt.so")
    if hook is not None:
        mod.set_axon_ntff_profile_hook(hook)


def kernel(x, edge_index, edge_weight, weight, bias):
    import os

    from concourse.bass_utils import run_bass_kernel_spmd

    x = np.asarray(x, dtype=np.float32)
    in_maps, NS = _prep_inputs(x, edge_index, edge_weight, weight, bias)
    nc = _get_nc(NS)
    trace = bool(int(os.environ.get("CHEB_TRACE", "0")))
    if trace:
        _ensure_ntff_hook()
    res = run_bass_kernel_spmd(nc, in_maps, list(range(NCORES)), trace=trace)
    _state["last_result"] = res
    out_T = np.concatenate([res.results[c]["out_t"] for c in range(NCORES)], axis=2)
    out = np.ascontiguousarray(
        out_T.transpose(0, 2, 1)[:, :N_NODES, :].astype(np.float32)
    )
    return out


# revision 11
# speedup vs baseline: 1.8433x; 1.2841x over previous
"""ChebConv (K=4) Trainium2 kernel — sparse gather-based SpMM.

Math (exactly matches the reference, which applies the spmm to `x` — not the
recurrence state — in every Chebyshev iteration):

    deg   = segment_sum(edge_weight, row)
    dinv  = deg^-1/2 (0 where deg <= 0)
    lap_e = -2*dinv[row]*edge_weight*dinv[col]      (edge part of L)
    Lx    = lap_e-SpMM(x) + 2*fill*x                (self loops)
    out   = x @ (W0 - W2) + Lx @ (W1 + 2*W2 + W3) + bias
          = x @ A' + (lap_e-SpMM(x)) @ B + bias,
            A' = W0 - W2 + 2*fill*B,  B = W1 + 2*W2 + W3

Device strategy: shard the 10240 (padded) destination rows over 8 cores
(10 dest-tiles of 128 rows each per core).  Per dest-tile, the ~2000 in-edges
are gathered edge-major with dma_gather (source-node rows of 512 bf16 = 1KB,
index i -> partition i%128), in 384-index single-packet chunks rotated over
the 4 SWDGE queues so Q7 descriptor generation overlaps SDMA drain.  NS
scatter matmuls S_j^T @ gathered_j accumulate the tile's Lx in PSUM; the
one-hot-by-dest S_j (lap values) are built on the otherwise-idle DVE from a
compact (dest,lap) table via tensor_scalar(iota is_equal dest) * lap.
Phase 1.5/2 transpose Lx per tile and apply the feature transforms in bf16;
outputs are written bf16 and upcast on host.
"""

import numpy as np
import ml_dtypes

B = 4
N_NODES = 10000
F = 128
SELF_LOOP_FILL = -0.05
NCORES = 8
NPAD = 10240                 # 80 tiles of 128; divisible by 8 cores
MROWS = NPAD // NCORES       # 1280 output rows per core
MT = MROWS // 128            # 10 dest-tiles per core
GT = NPAD // 128             # 80 global dest-tiles
BF = B * F                   # 512 features per gathered node row
CSLOT = 3                    # slot-tiles per gather chunk (384 idxs)
NQ = 4                       # SWDGE queues to rotate gather chunks over

_state = {}


def _build_nc(NS):
    """NS = slot-tiles (groups of 128 gathered edges) per dest-tile."""
    from contextlib import ExitStack

    import concourse.bass as bass
    import concourse.bacc as bacc
    import concourse.tile as tile
    from concourse import mybir, library_config

    dt = mybir.dt
    NS8 = NS * 8            # int16 idx columns per dest-tile (16-wrap)
    NI = NS * 128           # gathered rows per dest-tile
    nc = bacc.Bacc(
        "TRN2", target_bir_lowering=False, debug=False, num_devices=NCORES,
        num_swdge_queues=NQ,
    )

    xsrc = nc.declare_dram_parameter("xsrc", [N_NODES, BF], dt.bfloat16, isOutput=False)
    dstm = nc.declare_dram_parameter("dstm", [128, MT * NS], dt.bfloat16, isOutput=False)
    lapm = nc.declare_dram_parameter("lapm", [128, MT * NS], dt.bfloat16, isOutput=False)
    idxm = nc.declare_dram_parameter("idxm", [128, MT * NS8], dt.int16, isOutput=False)
    xt = nc.declare_dram_parameter("xt", [128, B, MROWS], dt.bfloat16, isOutput=False)
    wa = nc.declare_dram_parameter("wa", [128, 128], dt.bfloat16, isOutput=False)
    wb = nc.declare_dram_parameter("wb", [128, 128], dt.bfloat16, isOutput=False)
    biasv = nc.declare_dram_parameter("biasv", [128, 1], dt.float32, isOutput=False)
    ident = nc.declare_dram_parameter("ident", [128, 128], dt.float32, isOutput=False)
    iota = nc.declare_dram_parameter("iota", [128, NS * 128], dt.bfloat16, isOutput=False)
    out_t = nc.declare_dram_parameter(
        "out_t", [B, 128, MROWS], dt.bfloat16, isOutput=True
    )

    with ExitStack() as ctx:
        tc = ctx.enter_context(tile.TileContext(nc))
        nc.gpsimd.load_library(library_config.mlp)
        const = ctx.enter_context(tc.tile_pool(name="const", bufs=1))
        xgpool = ctx.enter_context(tc.tile_pool(name="xg", bufs=3))
        spool = ctx.enter_context(tc.tile_pool(name="smat", bufs=3))
        lxpool = ctx.enter_context(tc.tile_pool(name="lx", bufs=2))
        lxtpool = ctx.enter_context(tc.tile_pool(name="lxt", bufs=2))
        outpool = ctx.enter_context(tc.tile_pool(name="outstg", bufs=3))
        psum = ctx.enter_context(
            tc.tile_pool(name="psum", bufs=8, space=bass.MemorySpace.PSUM)
        )

        # idx table on sync (the first gather depends on it — issue early);
        # other constants on the scalar HWDGE queue
        idx_sb = const.tile([128, MT * NS8], dt.int16, tag="idx")
        nc.sync.dma_start(idx_sb[:], idxm[:])
        dst_sb = const.tile([128, MT * NS, 1], dt.bfloat16, tag="dst")
        nc.sync.dma_start(dst_sb[:, :, 0], dstm[:])
        lap_sb = const.tile([128, MT * NS, 1], dt.bfloat16, tag="lap")
        nc.sync.dma_start(lap_sb[:, :, 0], lapm[:])
        iota_sb = const.tile([128, NS, 128], dt.bfloat16, tag="iota")
        nc.scalar.dma_start(iota_sb[:, :, :], iota[:])
        id_sb = const.tile([128, 128], dt.float32, tag="ident")
        nc.scalar.dma_start(id_sb[:], ident[:])
        wa_sb = const.tile([128, 128], dt.bfloat16, tag="wa")
        nc.scalar.dma_start(wa_sb[:], wa[:])
        wb_sb = const.tile([128, 128], dt.bfloat16, tag="wb")
        nc.scalar.dma_start(wb_sb[:], wb[:])
        bias_sb = const.tile([128, 1], dt.float32, tag="bias")
        nc.scalar.dma_start(bias_sb[:], biasv[:])
        xt_sb = const.tile([128, B, MROWS], dt.bfloat16, tag="xt")
        nc.scalar.dma_start(xt_sb[:], xt[:])

        # PE warmup: dummy matmuls on the identity so the HAM clock-gate and
        # p-state ramp open before the first real gather lands.
        pw = psum.tile([128, 128], dt.float32, tag="ps", name="ps_warm")
        for i in range(36):
            nc.tensor.matmul(
                pw[:], id_sb[:], id_sb[:], start=(i == 0), stop=(i == 35)
            )

        q = 0
        for t in range(MT):
            # gather the tile's edge source rows: [128 e-lane, NS slot, 512],
            # in CSLOT-sized chunks rotating over the SWDGE queues
            xg = xgpool.tile([128, NS, BF], dt.bfloat16, tag="xg")
            for c0 in range(0, NS, CSLOT):
                cw = min(CSLOT, NS - c0)
                nc.gpsimd.dma_gather(
                    xg[:, c0 : c0 + cw, :],
                    xsrc[:],
                    idx_sb[:, t * NS8 + c0 * 8 : t * NS8 + (c0 + cw) * 8],
                    cw * 128, cw * 128, BF,
                    single_packet=True,
                    queue_num=q,
                )
                q = (q + 1) % NQ

            # build the scatter matrices on DVE: S_j[e,d] = lap * (d == dest)
            # two whole-tile broadcast ops instead of per-slot tensor_scalar
            st = spool.tile([128, NS, 128], dt.bfloat16, tag="st")
            nc.vector.tensor_tensor(
                out=st[:, :, :],
                in0=iota_sb[:, :, :],
                in1=dst_sb[:, t * NS : (t + 1) * NS, :].to_broadcast(
                    [128, NS, 128]
                ),
                op=mybir.AluOpType.is_equal,
            )
            nc.vector.tensor_tensor(
                out=st[:, :, :],
                in0=st[:, :, :],
                in1=lap_sb[:, t * NS : (t + 1) * NS, :].to_broadcast(
                    [128, NS, 128]
                ),
                op=mybir.AluOpType.mult,
            )

            ps = psum.tile([128, BF], dt.float32, tag="ps", name=f"ps1_{t}")
            for j in range(NS):
                nc.tensor.matmul(
                    ps[:],
                    st[:, j, :],
                    xg[:, j, :],
                    start=(j == 0),
                    stop=(j == NS - 1),
                )
            lx = lxpool.tile([128, BF], dt.float32, tag="lx")
            nc.vector.tensor_copy(lx[:], ps[:])

            # transpose Lx tile (node-major -> feature-major), per batch
            pt = psum.tile([128, BF], dt.float32, tag="ps", name=f"pt_{t}")
            for b in range(B):
                nc.tensor.transpose(
                    pt[:, b * 128 : (b + 1) * 128],
                    lx[:, b * 128 : (b + 1) * 128],
                    id_sb[:],
                )
            lxT = lxtpool.tile([128, BF], dt.bfloat16, tag="lxT")
            nc.vector.tensor_copy(lxT[:], pt[:])

            # phase 2: out_T = A'^T x^T + B^T Lx^T + bias (bf16 in, fp32 psum)
            ps2 = psum.tile([128, BF], dt.float32, tag="ps", name=f"ps2_{t}")
            for b in range(B):
                nc.tensor.matmul(
                    ps2[:, b * 128 : (b + 1) * 128],
                    wa_sb[:],
                    xt_sb[:, b, t * 128 : (t + 1) * 128],
                    start=True, stop=False,
                )
                nc.tensor.matmul(
                    ps2[:, b * 128 : (b + 1) * 128],
                    wb_sb[:],
                    lxT[:, b * 128 : (b + 1) * 128],
                    start=False, stop=True,
                )
            ot = outpool.tile([128, BF], dt.bfloat16, tag="ot")
            nc.scalar.activation(
                ot[:], ps2[:],
                mybir.ActivationFunctionType.Identity,
                bias=bias_sb[:],
            )
            for b in range(B):
                nc.scalar.dma_start(
                    out_t[b, :, t * 128 : (t + 1) * 128],
                    ot[:, b * 128 : (b + 1) * 128],
                )

    return nc


def _get_nc(NS):
    key = ("nc", NS)
    if key not in _state:
        nc = _build_nc(NS)
        nc.compile()
        _state[key] = nc
    return _state[key]


def _prep_inputs(x, edge_index, edge_weight, weight, bias):
    """Host-side graph preprocessing -> per-core device input maps + NS."""
    bf16 = ml_dtypes.bfloat16
    row = np.asarray(edge_index[0], dtype=np.int64)
    col = np.asarray(edge_index[1], dtype=np.int64)
    w = np.asarray(edge_weight, dtype=np.float32)

    deg = np.bincount(row, weights=w.astype(np.float64), minlength=N_NODES)
    deg = deg.astype(np.float32)
    dinv = np.where(deg > 0, np.where(deg > 0, deg, 1.0) ** -0.5, 0.0).astype(
        np.float32
    )
    lap2 = (-2.0 * dinv[row] * w * dinv[col]).astype(np.float32)

    # group edges by global dest-tile (row // 128)
    gt = row // 128
    order = np.argsort(gt, kind="stable")
    gts = gt[order]
    counts = np.bincount(gts, minlength=GT)
    starts = np.concatenate([[0], np.cumsum(counts)[:-1]])
    pos = np.arange(len(gts)) - starts[gts]          # position within tile
    NS = int(np.max((counts + 127) // 128))
    NS = max(NS, 1)
    NS8 = NS * 8
    NI = NS * 128

    e_col = col[order]
    e_lap = lap2[order]
    e_lane = pos % 128
    e_slot = pos // 128
    e_dst = (row[order] % 128).astype(np.int64)

    # per-tile padded index lists (pad with node 0; lap stays 0 -> S row 0)
    idx_full = np.zeros((GT, NI), dtype=np.int16)
    idx_full[gts, pos] = e_col.astype(np.int16)
    # compact scatter-matrix tables: dest lane + lap value per (tile, slot, lane)
    dst_all = np.zeros((GT, 128, NS), dtype=np.float32)
    lap_all = np.zeros((GT, 128, NS), dtype=np.float32)
    dst_all[gts, e_lane, e_slot] = e_dst.astype(np.float32)
    # padded lanes: dest 0 with lap 0 (contributes nothing)
    lap_all[gts, e_lane, e_slot] = e_lap

    # x in node-major (node, batch*feat) bf16 — the gather source
    xn = np.ascontiguousarray(
        np.transpose(np.asarray(x, np.float32), (1, 0, 2)).reshape(N_NODES, BF)
    )
    xsrc = xn.astype(bf16)
    xn_pad = np.zeros((NPAD, BF), dtype=np.float32)
    xn_pad[:N_NODES] = xn

    W = np.asarray(weight, dtype=np.float32)
    Bm = W[1] + 2.0 * W[2] + W[3]
    A = W[0] - W[2] + 2.0 * SELF_LOOP_FILL * Bm
    biasv = np.asarray(bias, dtype=np.float32).reshape(128, 1)
    identity = np.eye(128, dtype=np.float32)
    iota = np.broadcast_to(np.arange(128, dtype=np.float32), (128, NS, 128)).reshape(128, NS * 128)

    in_maps = []
    for c in range(NCORES):
        t0 = c * MT
        # idx wrap: idxm[p, t*NS8 + s] = idx_full[t0+t, s*16 + p%16]
        blk = idx_full[t0 : t0 + MT].reshape(MT, NS8, 16)       # (MT, s, 16)
        idxm16 = blk.transpose(2, 0, 1).reshape(16, MT * NS8)   # (16, MT*NS8)
        idxm = np.ascontiguousarray(np.tile(idxm16, (8, 1)))    # replicate
        dstm = np.ascontiguousarray(
            dst_all[t0 : t0 + MT].transpose(1, 0, 2).reshape(128, MT * NS)
        ).astype(bf16)
        lapm = np.ascontiguousarray(
            lap_all[t0 : t0 + MT].transpose(1, 0, 2).reshape(128, MT * NS)
        ).astype(bf16)
        r0 = c * MROWS
        xtc = np.ascontiguousarray(
            xn_pad[r0 : r0 + MROWS].reshape(MROWS, B, F).transpose(2, 1, 0)
        ).astype(bf16)
        in_maps.append(
            {
                "xsrc": xsrc,
                "dstm": dstm,
                "lapm": lapm,
                "idxm": idxm,
                "xt": xtc,
                "wa": A.astype(bf16),
                "wb": Bm.astype(bf16),
                "biasv": biasv,
                "ident": identity,
                "iota": np.ascontiguousarray(iota).astype(bf16),
            }
        )
    return in_maps, NS


def _ensure_ntff_hook():
    """Register the axon NTFF profiling hook if the image's antenv lacks it.

    The boot path degrades silently when ``antenv.axon_hooks`` is missing;
    recreate the tiny get/set holder and wire it to libaxon_pjrt.so so
    ``run_bass_kernel_spmd(trace=True)`` can capture NTFF profiles.
    """
    import sys
    import types

    try:
        from antenv.axon_hooks import get_axon_ntff_profile_hook  # noqa: F401

        return
    except ImportError:
        pass
    mod = types.ModuleType("antenv.axon_hooks")
    holder = {}
    mod.set_axon_ntff_profile_hook = lambda h: holder.__setitem__("h", h)
    mod.get_axon_ntff_profile_hook = lambda: holder.get("h")
    sys.modules["antenv.axon_hooks"] = mod
    import antenv

    antenv.axon_hooks = mod
    from trn_agent_boot.trn_boot import _ntff_profile_via_ctypes

    hook = _ntff_profile_via_ctypes("/opt/axon/libaxon_pjrt.so")
    if hook is not None:
        mod.set_axon_ntff_profile_hook(hook)


def kernel(x, edge_index, edge_weight, weight, bias):
    import os

    from concourse.bass_utils import run_bass_kernel_spmd

    x = np.asarray(x, dtype=np.float32)
    in_maps, NS = _prep_inputs(x, edge_index, edge_weight, weight, bias)
    nc = _get_nc(NS)
    trace = bool(int(os.environ.get("CHEB_TRACE", "0")))
    if trace:
        _ensure_ntff_hook()
    res = run_bass_kernel_spmd(nc, in_maps, list(range(NCORES)), trace=trace)
    _state["last_result"] = res
    out_T = np.concatenate([res.results[c]["out_t"] for c in range(NCORES)], axis=2)
    out = np.ascontiguousarray(
        out_T.transpose(0, 2, 1)[:, :N_NODES, :].astype(np.float32)
    )
    return out
